# revision 53
# baseline (speedup 1.0000x reference)
"""Trainium2 Bass kernel for nn_CLsLoss (ABCD soft-region weighted histograms +
profile likelihood).

Strategy (data-parallel over events, 8 cores):
  - Each core gets 1/8 of the 4M bkg events and 1/8 of the 4M sig events,
    reshaped to [128, COLS] (zero-padded weights for the tail). bkg and sig
    chunks are interleaved host-side so each on-device chunk of J columns
    holds J/2 bkg columns followed by J/2 sig columns and every elementwise
    op covers both datasets in one instruction.
  - Per event on-device: sigmoids s1,s2 (ScalarE, bias APs carry the runtime
    cuts); bin index idx = floor((mt-e0)/w) via one ScalarE affine with the
    HW round-to-nearest int16 conversion and a -0.5 bias.
  - Cumulative step slabs instead of one-hots: G_m = [idx >= 2m] for
    m = 0..25; the host recovers the pair histogram as G[m] - G[m+1].
    Steps are one op on every engine, balancing the pipeline: ~14-15
    is_ge on VectorE (4x mode), 5-6 on GpSimd (alternating per chunk),
    4-5 on ScalarE as saturated sigmoid(40*(idx-(2m-0.5))) (same table
    set as the region sigmoids, so no table reload), and slab 0
    ([idx >= 0] == 1) is a near-free GpSimd memset.
  - Payload: 8 slabs [d | d*s] where d = (w, w*s1, w*s2, w*s1*s2) in bf16
    and s = idx&1; the odd-half d*s is one broadcast tensor_tensor. Host
    recovers even bins as (all - odd).
  - Histogram via TensorE: per column t,
      psum[26, 8] += G[128, 26]^T @ SD[128, 8]
    accumulated over all columns per dataset in one PSUM fp32 group.
    (The hw weights AP allows only one free dim, so columns cannot be
    packed into a wider stationary.)
  - Host: sum per-core [26, 16] partials, difference the steps, map
    (m, odd) -> bin, derive regions A=H1-H12, B=H12, C=H-H1-H2+H12,
    D=H2-H12, scale by INT_LUMI, and evaluate the [50]-bin profile
    likelihood in float64.
"""

import os as _os

import numpy as np

NBIN = 50
N_EVENTS = 4_000_000
NCORES = 8
NPC = N_EVENTS // NCORES          # 500_000 events per core per dataset
P = 128
COLS = int(_os.environ.get("K_COLS", "3920"))   # 128*3920 >= NPC, tail w=0
JMAX = int(_os.environ.get("K_J", "1056"))      # joint chunk width (both ds)
Q = 26                            # pair index m = idx >> 1
NCH = 4                           # channels: w, w*s1, w*s2, w*s1*s2
NSLAB = 2 * NCH                   # payload slabs: [d | d*s]
PACK = 4                          # columns per matmul
QOH_GP = int(_os.environ.get("K_QOH_GP", "5"))  # step slabs on GpSimd
QOH_ACT = int(_os.environ.get("K_QOH_ACT", "5"))  # step slabs on ScalarE
QOH_BUFS = int(_os.environ.get("K_QOH_BUFS", "2"))
STEP_SLOPE = 40.0  # sigmoid(+-20) rounds to exactly 1/0 in bf16
INT_LUMI = 117100.0
EPS = 1e-6
STEEPNESS = 20.0


def _chunks():
    """Split per-dataset COLS into chunk widths of at most JMAX//2, each a
    multiple of PACK. The first chunk is small so the pipeline fills
    quickly (less serial head time)."""
    half = JMAX // 2
    ladder = [int(x) for x in
              _os.environ.get("K_LADDER", "80,288").split(",") if x]
    last = min(int(_os.environ.get("K_LAST", "64")), half)
    out = []
    c0 = 0
    while c0 < COLS:
        rem = COLS - c0
        if len(out) < len(ladder):
            w = min(ladder[len(out)], rem)
        elif rem > half + last:
            w = half
        elif rem > last:
            w = rem - last
        else:
            w = rem
        w = min(w, rem)
        assert w % PACK == 0 and w > 0, (w, COLS)
        out.append((c0, w))
        c0 += w
    return out


def _build_program():
    import concourse.bass as bass
    import concourse.bacc as bacc
    import concourse.mybir as mybir
    import concourse.tile as tile

    dt = mybir.dt
    Alu = mybir.AluOpType
    Act = mybir.ActivationFunctionType

    nc = bacc.Bacc("TRN2", target_bir_lowering=False, debug=False,
                   num_devices=NCORES)

    # Joint inputs: [:, 0:COLS] = bkg, [:, COLS:2*COLS] = sig, chunk-
    # interleaved by the host so one chunk slice is contiguous.
    names = ["f1", "f2", "mt", "w"]
    din = {n: nc.dram_tensor(n, [P, 2 * COLS], dt.float32,
                             kind="ExternalInput")
           for n in names}
    dpar = nc.dram_tensor("params", [P, 16], dt.float32, kind="ExternalInput")
    dout = nc.dram_tensor("hist_out", [Q, 2 * NSLAB], dt.float32,
                          kind="ExternalOutput")

    chunks = _chunks()

    from contextlib import ExitStack
    with tile.TileContext(nc) as tc, ExitStack() as ctx:
        io_pool = ctx.enter_context(tc.tile_pool(name="io", bufs=2))
        act_pool = ctx.enter_context(tc.tile_pool(
            name="acto", bufs=int(_os.environ.get("K_ACT_BUFS", "2"))))
        sd_pool = ctx.enter_context(tc.tile_pool(name="sd", bufs=2))
        qoh_pool = ctx.enter_context(tc.tile_pool(name="qoh", bufs=QOH_BUFS))
        const_pool = ctx.enter_context(tc.tile_pool(name="const", bufs=1))
        psum_pool = ctx.enter_context(
            tc.tile_pool(name="psum", bufs=1, space=bass.MemorySpace.PSUM))
        out_pool = ctx.enter_context(tc.tile_pool(name="out", bufs=1))

        par = const_pool.tile([P, 16], dt.float32)
        warm = const_pool.tile([P, 16], dt.bfloat16)
        warm_src = const_pool.tile([P, 16], dt.bfloat16)
        # memset has no input deps, so the sigmoid table load starts at t=0
        # and fully overlaps the params + first-chunk DMAs
        nc.gpsimd.memset(warm_src[:], 0.0)
        # params ride the ACT DGE ring so the first mt chunk is not queued
        # behind them on the SP ring
        nc.scalar.dma_start(par[:], dpar[:])
        bias1 = par[:, 0:1]     # -20*cut1
        bias2 = par[:, 1:2]     # -20*cut2
        invw = par[:, 2:3]      # 1/bin_width
        nege0h = par[:, 3:4]    # -edges[0]/bin_width - 0.5  (floor via rint)
        # touch Sigmoid early so the ACT table set loads during input DMA
        nc.scalar.activation(warm[:], warm_src[:], Act.Sigmoid)

        ps = {ds: psum_pool.tile([Q, NSLAB], dt.float32,
                                 name=f"ps_{ds}", tag=f"ps_{ds}")
              for ds in ("bkg", "sig")}
        started = {"bkg": False, "sig": False}
        n_packs = {"bkg": COLS, "sig": COLS}
        done_packs = {"bkg": 0, "sig": 0}

        for ci, (c0, w) in enumerate(chunks):
            J = 2 * w  # joint width: w bkg cols + w sig cols
            f1 = io_pool.tile([P, J], dt.float32, tag="f1")
            f2 = io_pool.tile([P, J], dt.float32, tag="f2")
            mt = io_pool.tile([P, J], dt.float32, tag="mt")
            wt = io_pool.tile([P, J], dt.float32, tag="w")
            # host interleaves so joint chunk k occupies cols [2*c0, 2*c0+J);
            # mt first: it feeds idx16 -> q2 -> all one-hot slabs
            nc.sync.dma_start(mt[:], din["mt"][:, 2 * c0:2 * c0 + J])
            nc.sync.dma_start(f1[:], din["f1"][:, 2 * c0:2 * c0 + J])
            nc.sync.dma_start(f2[:], din["f2"][:, 2 * c0:2 * c0 + J])
            nc.sync.dma_start(wt[:], din["w"][:, 2 * c0:2 * c0 + J])

            s12 = act_pool.tile([P, 2 * J], dt.bfloat16, tag="s12")
            s1 = s12[:, 0:J]
            s2 = s12[:, J:2 * J]
            idx16 = act_pool.tile([P, J], dt.int16, tag="idx16")
            # int16 (not bf16): the hw TSP bitvec path cannot cast dtypes
            s16 = act_pool.tile([P, J], dt.int16, tag="s16")
            sd = sd_pool.tile([P, NSLAB * J], dt.bfloat16, tag="sd")

            # idx = floor((mt - e0)/binw) via rint(x - 0.5) on ACT -> int16;
            # first so q2 and the slab engines start as early as possible
            nc.scalar.activation(idx16[:], mt[:], Act.Identity,
                                 bias=nege0h, scale=invw)
            nc.scalar.activation(s1, f1[:], Act.Sigmoid,
                                 bias=bias1, scale=STEEPNESS)
            nc.scalar.activation(s2, f2[:], Act.Sigmoid,
                                 bias=bias2, scale=STEEPNESS)
            # d slab 0 = w in bf16 (ACT copy straight into the payload tile)
            nc.scalar.activation(sd[:, 0:J], wt[:], Act.Copy)

            # s = idx & 1 (odd/even split of each bin pair)
            nc.vector.tensor_scalar(s16[:], idx16[:], 1, None,
                                    Alu.bitwise_and)
            # d channels
            # (d1|d2) = w*(s1|s2) in one broadcast TT, then d3 = d1*s2
            w_b = sd[:, 0:J].rearrange("p (o t) -> p o t", o=1)
            w_b = w_b.to_broadcast((P, 2, J))
            nc.vector.tensor_tensor(
                sd[:, J:3 * J].rearrange("p (r t) -> p r t", r=2),
                w_b, s12[:].rearrange("p (r t) -> p r t", r=2), Alu.mult)
            nc.vector.tensor_tensor(sd[:, 3 * J:4 * J], sd[:, J:2 * J],
                                    s2, Alu.mult)
            # odd half: d * s  (one broadcast TT over 4J)
            d_b = sd[:, 0:NCH * J].rearrange("p (r t) -> p r t", r=NCH)
            s_b = s16[:].rearrange("p (o t) -> p o t", o=1)
            s_b = s_b.to_broadcast((P, NCH, J))
            nc.vector.tensor_tensor(
                sd[:, NCH * J:2 * NCH * J].rearrange("p (r t) -> p r t",
                                                     r=NCH),
                s_b, d_b, Alu.mult)

            # cumulative step slabs: G_m[i] = [idx_i >= 2m]. The host
            # recovers pair histograms as G[m] - G[m+1]. Steps are one op
            # everywhere: is_ge on Vector/GpSimd, a saturated sigmoid on
            # ScalarE (same table set as s1/s2), and slab 0 ([idx >= 0]) is
            # a near-free GpSimd memset of ones.
            qoh = qoh_pool.tile([P, Q * J], dt.bfloat16, tag="qoh")
            n_base = Q - QOH_GP - 1 - QOH_ACT  # ACT slabs sit below Pool's
            # alternate one slab between Pool and DVE to hit the fractional
            # engine-balance optimum
            alt = int(_os.environ.get("K_ALT", "1"))
            gp = QOH_GP + (1 if (alt and ci % 2 == 1) else 0)
            n_act = QOH_ACT - (1 if (alt and ci % 2 == 1) else 0)
            act_set = set(range(n_base, n_base + n_act))
            pool_set = set(range(Q - gp, Q)) - act_set
            for m in range(Q):
                slab = qoh[:, m * J:(m + 1) * J]
                if m == 0:
                    nc.gpsimd.memset(slab, 1.0)
                elif m in act_set:
                    i = m - n_base
                    negm = par[:, 4 + i:5 + i]  # -SLOPE*(2m - 0.5)
                    nc.scalar.activation(slab, idx16[:], Act.Sigmoid,
                                         bias=negm, scale=STEP_SLOPE)
                elif m in pool_set:
                    nc.gpsimd.tensor_scalar(slab, idx16[:], float(2 * m),
                                            None, Alu.is_ge)
                else:
                    nc.vector.tensor_scalar(slab, idx16[:], float(2 * m),
                                            None, Alu.is_ge)

            # per-column matmuls (hw: the weights AP allows only one free
            # dim, so columns cannot be packed into a wider stationary)
            qoh_r = qoh[:].rearrange("p (m t) -> p t m", t=J)
            sd_r = sd[:].rearrange("p (j t) -> p t j", t=J)
            for t0 in range(J):
                ds = "bkg" if t0 < w else "sig"
                first = not started[ds]
                started[ds] = True
                done_packs[ds] += 1
                last = done_packs[ds] == n_packs[ds]
                nc.tensor.matmul(
                    ps[ds][:], qoh_r[:, t0, :], sd_r[:, t0, :],
                    start=first, stop=last, skip_group_check=True)

        out_sb = out_pool.tile([Q, 2 * NSLAB], dt.float32)
        nc.vector.tensor_copy(out_sb[:, 0:NSLAB], ps["bkg"][:])
        nc.vector.tensor_copy(out_sb[:, NSLAB:], ps["sig"][:])
        nc.sync.dma_start(dout[:], out_sb[:])

    nc.compile()
    return nc


def _shard_joint(arr: np.ndarray, core: int, chunks) -> np.ndarray:
    """arr: (bkg_full, sig_full) pair -> [P, 2*COLS] chunk-interleaved."""
    bkg_full, sig_full = arr
    out = np.zeros((P, 2 * COLS), dtype=np.float32)
    halves = []
    for full in (bkg_full, sig_full):
        sl = full[core * NPC:(core + 1) * NPC]
        h = np.zeros(P * COLS, dtype=np.float32)
        h[:NPC] = sl
        halves.append(h.reshape(P, COLS))
    b, s = halves
    for c0, w in chunks:
        out[:, 2 * c0:2 * c0 + w] = b[:, c0:c0 + w]
        out[:, 2 * c0 + w:2 * c0 + 2 * w] = s[:, c0:c0 + w]
    return out


def _decode(block: np.ndarray) -> np.ndarray:
    """[Q, NSLAB] psum block of cumulative steps -> [NBIN, NCH] histogram.

    Row m holds G[m] = sum over events with idx >= 2m; pair m is
    G[m] - G[m+1] (G[26] = 0)."""
    pair = block.astype(np.float64).copy()
    pair[:-1] -= block[1:]
    h_all = pair[:, 0:NCH]
    h_odd = pair[:, NCH:2 * NCH]
    h_even = h_all - h_odd
    bins = np.empty((2 * Q, NCH))
    bins[0::2] = h_even
    bins[1::2] = h_odd
    return bins[:NBIN]


def _regions(h: np.ndarray) -> np.ndarray:
    """[NBIN, 4] channel hist (H, H1, H2, H12) -> regions (A,B,C,D)*lumi."""
    H, H1, H2, H12 = h[:, 0], h[:, 1], h[:, 2], h[:, 3]
    A = H1 - H12
    B = H12
    C = H - H1 - H2 + H12
    D = H2 - H12
    return np.stack([A, B, C, D], axis=-1) * INT_LUMI


def _likelihood(hb: np.ndarray, hs: np.ndarray) -> float:
    """hb/hs: [NBIN, 4] region histograms (A,B,C,D) in float64."""
    from scipy.special import gammaln

    obs_A, obs_B, obs_C, obs_D = hb[:, 0], hb[:, 1], hb[:, 2], hb[:, 3]
    S_A, S_B, S_C, S_D = hs[:, 0], hs[:, 1], hs[:, 2], hs[:, 3]
    mu = 1.0
    # theta = 0, nA/nC/nD = obs_A/obs_C/obs_D
    exp_A = obs_A + mu * S_A
    exp_C = obs_C + mu * S_C
    exp_D = obs_D + mu * S_D
    # (1 + delta) ** theta == 1 at theta = 0
    bkg_SR = obs_A * obs_D / (obs_C + EPS)
    exp_B = bkg_SR + mu * S_B

    def pois(o, e):
        return o * np.log(e + EPS) - e - gammaln(o + 1.0)

    llh = (pois(obs_A, exp_A) + pois(obs_B, exp_B)
           + pois(obs_C, exp_C) + pois(obs_D, exp_D))
    return -float(llh.sum())


_NC_CACHE = None
LAST_RESULTS = None


def kernel(f1_bkg, f2_bkg, mt_bkg, w_bkg, f1_sig, f2_sig, mt_sig, w_sig,
           cut1, cut2, mt_bin_edges):
    global _NC_CACHE, LAST_RESULTS
    from concourse.bass_utils import run_bass_kernel_spmd

    if _NC_CACHE is None:
        _NC_CACHE = _build_program()
    nc = _NC_CACHE

    edges = np.asarray(mt_bin_edges, dtype=np.float64)
    width = float(edges[1] - edges[0])
    e0 = float(edges[0])
    par = np.zeros((P, 16), dtype=np.float32)
    par[:, 0] = -STEEPNESS * float(cut1)
    par[:, 1] = -STEEPNESS * float(cut2)
    par[:, 2] = 1.0 / width
    par[:, 3] = -e0 / width - 0.5
    n_base = Q - QOH_GP - 1 - QOH_ACT
    for i in range(QOH_ACT):
        # ACT step slab: sigmoid(SLOPE*(idx - (2m - 0.5))) == [idx >= 2m]
        par[:, 4 + i] = -STEP_SLOPE * (2.0 * (n_base + i) - 0.5)

    pairs = {
        "f1": (np.asarray(f1_bkg, np.float32), np.asarray(f1_sig, np.float32)),
        "f2": (np.asarray(f2_bkg, np.float32), np.asarray(f2_sig, np.float32)),
        "mt": (np.asarray(mt_bkg, np.float32), np.asarray(mt_sig, np.float32)),
        "w": (np.asarray(w_bkg, np.float32), np.asarray(w_sig, np.float32)),
    }
    chunks = _chunks()

    in_maps = []
    for core in range(NCORES):
        m = {k: _shard_joint(v, core, chunks) for k, v in pairs.items()}
        m["params"] = par
        in_maps.append(m)

    try:
        res = run_bass_kernel_spmd(nc, in_maps, core_ids=list(range(NCORES)))
    except Exception:
        # transient device states (e.g. a wedged exec unit from a prior run)
        # typically clear on retry
        res = run_bass_kernel_spmd(nc, in_maps, core_ids=list(range(NCORES)))
    LAST_RESULTS = res

    total = np.zeros((Q, 2 * NSLAB), dtype=np.float64)
    for rmap in res.results:
        total += rmap["hist_out"].astype(np.float64)

    hb = _regions(_decode(total[:, 0:NSLAB]))
    hs = _regions(_decode(total[:, NSLAB:]))
    out = _likelihood(hb, hs)
    return np.float32(out)


# revision 61
# speedup vs baseline: 1.0008x; 1.0008x over previous
"""Trainium2 Bass kernel for nn_CLsLoss (ABCD soft-region weighted histograms +
profile likelihood).

Strategy (data-parallel over events, 8 cores):
  - Each core gets 1/8 of the 4M bkg events and 1/8 of the 4M sig events,
    reshaped to [128, COLS] (zero-padded weights for the tail). bkg and sig
    chunks are interleaved host-side so each on-device chunk of J columns
    holds J/2 bkg columns followed by J/2 sig columns and every elementwise
    op covers both datasets in one instruction.
  - Per event on-device: sigmoids s1,s2 (ScalarE, bias APs carry the runtime
    cuts); bin index idx = floor((mt-e0)/w) via one ScalarE affine with the
    HW round-to-nearest int16 conversion and a -0.5 bias.
  - Cumulative step slabs instead of one-hots: G_m = [idx >= 2m] for
    m = 0..25; the host recovers the pair histogram as G[m] - G[m+1].
    Steps are one op on every engine, balancing the pipeline: ~14-15
    is_ge on VectorE (4x mode), 5-6 on GpSimd (alternating per chunk),
    4-5 on ScalarE as saturated sigmoid(40*(idx-(2m-0.5))) (same table
    set as the region sigmoids, so no table reload), and slab 0
    ([idx >= 0] == 1) is a near-free GpSimd memset.
  - Payload: 8 slabs [d | d*s] where d = (w, w*s1, w*s2, w*s1*s2) in bf16
    and s = idx&1; the odd-half d*s is one broadcast tensor_tensor. Host
    recovers even bins as (all - odd).
  - Histogram via TensorE: per column t,
      psum[26, 8] += G[128, 26]^T @ SD[128, 8]
    accumulated over all columns per dataset in one PSUM fp32 group.
    (The hw weights AP allows only one free dim, so columns cannot be
    packed into a wider stationary.)
  - Host: sum per-core [26, 16] partials, difference the steps, map
    (m, odd) -> bin, derive regions A=H1-H12, B=H12, C=H-H1-H2+H12,
    D=H2-H12, scale by INT_LUMI, and evaluate the [50]-bin profile
    likelihood in float64.
"""

import os as _os

import numpy as np

NBIN = 50
N_EVENTS = 4_000_000
NCORES = 8
NPC = N_EVENTS // NCORES          # 500_000 events per core per dataset
P = 128
COLS = int(_os.environ.get("K_COLS", "3920"))   # 128*3920 >= NPC, tail w=0
JMAX = int(_os.environ.get("K_J", "1056"))      # joint chunk width (both ds)
Q = 26                            # pair index m = idx >> 1
NCH = 4                           # channels: w, w*s1, w*s2, w*s1*s2
NSLAB = 2 * NCH                   # payload slabs: [d | d*s]
PACK = 4                          # columns per matmul
QOH_GP = int(_os.environ.get("K_QOH_GP", "5"))  # step slabs on GpSimd
QOH_ACT = int(_os.environ.get("K_QOH_ACT", "5"))  # step slabs on ScalarE
QOH_BUFS = int(_os.environ.get("K_QOH_BUFS", "2"))
STEP_SLOPE = 40.0  # sigmoid(+-20) rounds to exactly 1/0 in bf16
INT_LUMI = 117100.0
EPS = 1e-6
STEEPNESS = 20.0


def _chunks():
    """Split per-dataset COLS into chunk widths of at most JMAX//2, each a
    multiple of PACK. The first chunk is small so the pipeline fills
    quickly (less serial head time)."""
    half = JMAX // 2
    ladder = [int(x) for x in
              _os.environ.get("K_LADDER", "84,288").split(",") if x]
    last = min(int(_os.environ.get("K_LAST", "64")), half)
    out = []
    c0 = 0
    while c0 < COLS:
        rem = COLS - c0
        if len(out) < len(ladder):
            w = min(ladder[len(out)], rem)
        elif rem > half + last:
            w = half
        elif rem > last:
            w = rem - last
        else:
            w = rem
        w = min(w, rem)
        assert w % PACK == 0 and w > 0, (w, COLS)
        out.append((c0, w))
        c0 += w
    return out


def _build_program():
    import concourse.bass as bass
    import concourse.bacc as bacc
    import concourse.mybir as mybir
    import concourse.tile as tile

    dt = mybir.dt
    Alu = mybir.AluOpType
    Act = mybir.ActivationFunctionType

    nc = bacc.Bacc("TRN2", target_bir_lowering=False, debug=False,
                   num_devices=NCORES)

    # Joint inputs: [:, 0:COLS] = bkg, [:, COLS:2*COLS] = sig, chunk-
    # interleaved by the host so one chunk slice is contiguous.
    names = ["f1", "f2", "mt", "w"]
    din = {n: nc.dram_tensor(n, [P, 2 * COLS], dt.float32,
                             kind="ExternalInput")
           for n in names}
    dpar = nc.dram_tensor("params", [P, 16], dt.float32, kind="ExternalInput")
    dout = nc.dram_tensor("hist_out", [Q, 2 * NSLAB], dt.float32,
                          kind="ExternalOutput")

    chunks = _chunks()

    from contextlib import ExitStack
    with tile.TileContext(nc) as tc, ExitStack() as ctx:
        io_pool = ctx.enter_context(tc.tile_pool(name="io", bufs=2))
        act_pool = ctx.enter_context(tc.tile_pool(
            name="acto", bufs=int(_os.environ.get("K_ACT_BUFS", "2"))))
        sd_pool = ctx.enter_context(tc.tile_pool(name="sd", bufs=2))
        qoh_pool = ctx.enter_context(tc.tile_pool(name="qoh", bufs=QOH_BUFS))
        const_pool = ctx.enter_context(tc.tile_pool(name="const", bufs=1))
        psum_pool = ctx.enter_context(
            tc.tile_pool(name="psum", bufs=1, space=bass.MemorySpace.PSUM))
        out_pool = ctx.enter_context(tc.tile_pool(name="out", bufs=1))

        par = const_pool.tile([P, 16], dt.float32)
        warm = const_pool.tile([P, 16], dt.bfloat16)
        warm_src = const_pool.tile([P, 16], dt.bfloat16)
        # memset has no input deps, so the sigmoid table load starts at t=0
        # and fully overlaps the params + first-chunk DMAs
        nc.gpsimd.memset(warm_src[:], 0.0)
        # params ride the ACT DGE ring so the first mt chunk is not queued
        # behind them on the SP ring
        nc.scalar.dma_start(par[:], dpar[:])
        bias1 = par[:, 0:1]     # -20*cut1
        bias2 = par[:, 1:2]     # -20*cut2
        invw = par[:, 2:3]      # 1/bin_width
        nege0h = par[:, 3:4]    # -edges[0]/bin_width - 0.5  (floor via rint)
        # touch Sigmoid early so the ACT table set loads during input DMA
        nc.scalar.activation(warm[:], warm_src[:], Act.Sigmoid)

        ps = {ds: psum_pool.tile([Q, NSLAB], dt.float32,
                                 name=f"ps_{ds}", tag=f"ps_{ds}")
              for ds in ("bkg", "sig")}
        started = {"bkg": False, "sig": False}
        n_packs = {"bkg": COLS, "sig": COLS}
        done_packs = {"bkg": 0, "sig": 0}

        for ci, (c0, w) in enumerate(chunks):
            J = 2 * w  # joint width: w bkg cols + w sig cols
            f1 = io_pool.tile([P, J], dt.float32, tag="f1")
            f2 = io_pool.tile([P, J], dt.float32, tag="f2")
            mt = io_pool.tile([P, J], dt.float32, tag="mt")
            wt = io_pool.tile([P, J], dt.float32, tag="w")
            # host interleaves so joint chunk k occupies cols [2*c0, 2*c0+J);
            # mt first: it feeds idx16 -> q2 -> all one-hot slabs
            nc.sync.dma_start(mt[:], din["mt"][:, 2 * c0:2 * c0 + J])
            nc.sync.dma_start(f1[:], din["f1"][:, 2 * c0:2 * c0 + J])
            nc.sync.dma_start(f2[:], din["f2"][:, 2 * c0:2 * c0 + J])
            nc.sync.dma_start(wt[:], din["w"][:, 2 * c0:2 * c0 + J])

            s12 = act_pool.tile([P, 2 * J], dt.bfloat16, tag="s12")
            s1 = s12[:, 0:J]
            s2 = s12[:, J:2 * J]
            idx16 = act_pool.tile([P, J], dt.int16, tag="idx16")
            # int16 (not bf16): the hw TSP bitvec path cannot cast dtypes
            s16 = act_pool.tile([P, J], dt.int16, tag="s16")
            sd = sd_pool.tile([P, NSLAB * J], dt.bfloat16, tag="sd")

            # idx = floor((mt - e0)/binw) via rint(x - 0.5) on ACT -> int16;
            # first so q2 and the slab engines start as early as possible
            nc.scalar.activation(idx16[:], mt[:], Act.Identity,
                                 bias=nege0h, scale=invw)
            nc.scalar.activation(s1, f1[:], Act.Sigmoid,
                                 bias=bias1, scale=STEEPNESS)
            nc.scalar.activation(s2, f2[:], Act.Sigmoid,
                                 bias=bias2, scale=STEEPNESS)
            # d slab 0 = w in bf16 (ACT copy straight into the payload tile)
            nc.scalar.activation(sd[:, 0:J], wt[:], Act.Copy)

            # s = idx & 1 (odd/even split of each bin pair)
            nc.vector.tensor_scalar(s16[:], idx16[:], 1, None,
                                    Alu.bitwise_and)
            # d channels
            # (d1|d2) = w*(s1|s2) in one broadcast TT, then d3 = d1*s2
            w_b = sd[:, 0:J].rearrange("p (o t) -> p o t", o=1)
            w_b = w_b.to_broadcast((P, 2, J))
            nc.vector.tensor_tensor(
                sd[:, J:3 * J].rearrange("p (r t) -> p r t", r=2),
                w_b, s12[:].rearrange("p (r t) -> p r t", r=2), Alu.mult)
            nc.vector.tensor_tensor(sd[:, 3 * J:4 * J], sd[:, J:2 * J],
                                    s2, Alu.mult)
            # odd half: d * s  (one broadcast TT over 4J)
            d_b = sd[:, 0:NCH * J].rearrange("p (r t) -> p r t", r=NCH)
            s_b = s16[:].rearrange("p (o t) -> p o t", o=1)
            s_b = s_b.to_broadcast((P, NCH, J))
            nc.vector.tensor_tensor(
                sd[:, NCH * J:2 * NCH * J].rearrange("p (r t) -> p r t",
                                                     r=NCH),
                s_b, d_b, Alu.mult)

            # cumulative step slabs: G_m[i] = [idx_i >= 2m]. The host
            # recovers pair histograms as G[m] - G[m+1]. Steps are one op
            # everywhere: is_ge on Vector/GpSimd, a saturated sigmoid on
            # ScalarE (same table set as s1/s2), and slab 0 ([idx >= 0]) is
            # a near-free GpSimd memset of ones.
            qoh = qoh_pool.tile([P, Q * J], dt.bfloat16, tag="qoh")
            n_base = Q - QOH_GP - 1 - QOH_ACT  # ACT slabs sit below Pool's
            # alternate one slab between Pool and DVE to hit the fractional
            # engine-balance optimum
            alt = int(_os.environ.get("K_ALT", "1"))
            gp = QOH_GP + (1 if (alt and ci % 2 == 1) else 0)
            n_act = QOH_ACT - (1 if (alt and ci % 2 == 1) else 0)
            act_set = set(range(n_base, n_base + n_act))
            pool_set = set(range(Q - gp, Q)) - act_set
            for m in range(Q):
                slab = qoh[:, m * J:(m + 1) * J]
                if m == 0:
                    nc.gpsimd.memset(slab, 1.0)
                elif m in act_set:
                    i = m - n_base
                    negm = par[:, 4 + i:5 + i]  # -SLOPE*(2m - 0.5)
                    nc.scalar.activation(slab, idx16[:], Act.Sigmoid,
                                         bias=negm, scale=STEP_SLOPE)
                elif m in pool_set:
                    nc.gpsimd.tensor_scalar(slab, idx16[:], float(2 * m),
                                            None, Alu.is_ge)
                else:
                    nc.vector.tensor_scalar(slab, idx16[:], float(2 * m),
                                            None, Alu.is_ge)

            # per-column matmuls (hw: the weights AP allows only one free
            # dim, so columns cannot be packed into a wider stationary)
            qoh_r = qoh[:].rearrange("p (m t) -> p t m", t=J)
            sd_r = sd[:].rearrange("p (j t) -> p t j", t=J)
            for t0 in range(J):
                ds = "bkg" if t0 < w else "sig"
                first = not started[ds]
                started[ds] = True
                done_packs[ds] += 1
                last = done_packs[ds] == n_packs[ds]
                nc.tensor.matmul(
                    ps[ds][:], qoh_r[:, t0, :], sd_r[:, t0, :],
                    start=first, stop=last, skip_group_check=True)

        out_sb = out_pool.tile([Q, 2 * NSLAB], dt.float32)
        nc.vector.tensor_copy(out_sb[:, 0:NSLAB], ps["bkg"][:])
        nc.vector.tensor_copy(out_sb[:, NSLAB:], ps["sig"][:])
        nc.sync.dma_start(dout[:], out_sb[:])

    nc.compile()
    return nc


def _shard_joint(arr: np.ndarray, core: int, chunks) -> np.ndarray:
    """arr: (bkg_full, sig_full) pair -> [P, 2*COLS] chunk-interleaved."""
    bkg_full, sig_full = arr
    out = np.zeros((P, 2 * COLS), dtype=np.float32)
    halves = []
    for full in (bkg_full, sig_full):
        sl = full[core * NPC:(core + 1) * NPC]
        h = np.zeros(P * COLS, dtype=np.float32)
        h[:NPC] = sl
        halves.append(h.reshape(P, COLS))
    b, s = halves
    for c0, w in chunks:
        out[:, 2 * c0:2 * c0 + w] = b[:, c0:c0 + w]
        out[:, 2 * c0 + w:2 * c0 + 2 * w] = s[:, c0:c0 + w]
    return out


def _decode(block: np.ndarray) -> np.ndarray:
    """[Q, NSLAB] psum block of cumulative steps -> [NBIN, NCH] histogram.

    Row m holds G[m] = sum over events with idx >= 2m; pair m is
    G[m] - G[m+1] (G[26] = 0)."""
    pair = block.astype(np.float64).copy()
    pair[:-1] -= block[1:]
    h_all = pair[:, 0:NCH]
    h_odd = pair[:, NCH:2 * NCH]
    h_even = h_all - h_odd
    bins = np.empty((2 * Q, NCH))
    bins[0::2] = h_even
    bins[1::2] = h_odd
    return bins[:NBIN]


def _regions(h: np.ndarray) -> np.ndarray:
    """[NBIN, 4] channel hist (H, H1, H2, H12) -> regions (A,B,C,D)*lumi."""
    H, H1, H2, H12 = h[:, 0], h[:, 1], h[:, 2], h[:, 3]
    A = H1 - H12
    B = H12
    C = H - H1 - H2 + H12
    D = H2 - H12
    return np.stack([A, B, C, D], axis=-1) * INT_LUMI


def _likelihood(hb: np.ndarray, hs: np.ndarray) -> float:
    """hb/hs: [NBIN, 4] region histograms (A,B,C,D) in float64."""
    from scipy.special import gammaln

    obs_A, obs_B, obs_C, obs_D = hb[:, 0], hb[:, 1], hb[:, 2], hb[:, 3]
    S_A, S_B, S_C, S_D = hs[:, 0], hs[:, 1], hs[:, 2], hs[:, 3]
    mu = 1.0
    # theta = 0, nA/nC/nD = obs_A/obs_C/obs_D
    exp_A = obs_A + mu * S_A
    exp_C = obs_C + mu * S_C
    exp_D = obs_D + mu * S_D
    # (1 + delta) ** theta == 1 at theta = 0
    bkg_SR = obs_A * obs_D / (obs_C + EPS)
    exp_B = bkg_SR + mu * S_B

    def pois(o, e):
        return o * np.log(e + EPS) - e - gammaln(o + 1.0)

    llh = (pois(obs_A, exp_A) + pois(obs_B, exp_B)
           + pois(obs_C, exp_C) + pois(obs_D, exp_D))
    return -float(llh.sum())


_NC_CACHE = None
LAST_RESULTS = None


def kernel(f1_bkg, f2_bkg, mt_bkg, w_bkg, f1_sig, f2_sig, mt_sig, w_sig,
           cut1, cut2, mt_bin_edges):
    global _NC_CACHE, LAST_RESULTS
    from concourse.bass_utils import run_bass_kernel_spmd

    if _NC_CACHE is None:
        _NC_CACHE = _build_program()
    nc = _NC_CACHE

    edges = np.asarray(mt_bin_edges, dtype=np.float64)
    width = float(edges[1] - edges[0])
    e0 = float(edges[0])
    par = np.zeros((P, 16), dtype=np.float32)
    par[:, 0] = -STEEPNESS * float(cut1)
    par[:, 1] = -STEEPNESS * float(cut2)
    par[:, 2] = 1.0 / width
    par[:, 3] = -e0 / width - 0.5
    n_base = Q - QOH_GP - 1 - QOH_ACT
    for i in range(QOH_ACT):
        # ACT step slab: sigmoid(SLOPE*(idx - (2m - 0.5))) == [idx >= 2m]
        par[:, 4 + i] = -STEP_SLOPE * (2.0 * (n_base + i) - 0.5)

    pairs = {
        "f1": (np.asarray(f1_bkg, np.float32), np.asarray(f1_sig, np.float32)),
        "f2": (np.asarray(f2_bkg, np.float32), np.asarray(f2_sig, np.float32)),
        "mt": (np.asarray(mt_bkg, np.float32), np.asarray(mt_sig, np.float32)),
        "w": (np.asarray(w_bkg, np.float32), np.asarray(w_sig, np.float32)),
    }
    chunks = _chunks()

    in_maps = []
    for core in range(NCORES):
        m = {k: _shard_joint(v, core, chunks) for k, v in pairs.items()}
        m["params"] = par
        in_maps.append(m)

    try:
        res = run_bass_kernel_spmd(nc, in_maps, core_ids=list(range(NCORES)))
    except Exception:
        # transient device states (e.g. a wedged exec unit from a prior run)
        # typically clear on retry
        res = run_bass_kernel_spmd(nc, in_maps, core_ids=list(range(NCORES)))
    LAST_RESULTS = res

    total = np.zeros((Q, 2 * NSLAB), dtype=np.float64)
    for rmap in res.results:
        total += rmap["hist_out"].astype(np.float64)

    hb = _regions(_decode(total[:, 0:NSLAB]))
    hs = _regions(_decode(total[:, NSLAB:]))
    out = _likelihood(hb, hs)
    return np.float32(out)
